# revision 21
# baseline (speedup 1.0000x reference)
"""Trainium2 Bass kernel for nn_K_Rectify (gnn message passing, idw + rmsnorm).

Reference computation (B=128, NTOT=129, N=128, GS=16, C=384):
    x   = f[:, 1:, :]                         # [B, N, C]
    nf  = x.reshape(B*N, C)[idx]              # [B, N, GS, C] gather (global flat idx)
    w   = 1/(dist+eps); w /= w.sum(-1)        # idw weights
    sf  = sum_g w * (nf - x) = (sum_g w*nf) - x    (weights sum to 1)
    out = (rf[1:] + x) + rmsnorm(sf) * knorm_w
    cat cls token back on.

Sharding: data-parallel over batch B across 8 cores (16 batches / core).
idx values index the full flattened [B*N] table, so the gather source
table is replicated to every core; everything else is sharded.

v2: the gather table, gathered neighbors, and the whole weighted-sum
path run in bf16 (rel tol is 2e-2; bf16 error ~0.5%). This halves the
dominant HBM/DMA-bus cost (the 32k x 1536B random gather per core) and
makes the PE diag-matmuls single-pass. The 16 neighbor groups are split
PE/DVE/ACT; the DVE/ACT partial sum (minus the center x) re-enters the
PE PSUM accumulator through one extra identity matmul so no f32 adds
are needed on DVE. idw weights are normalized host-side (elementwise,
0.4% of FLOPs) and shipped as a tiny bf16 tensor.
"""

import sys

sys.path.insert(0, "/opt/trn_rl_repo")

import ml_dtypes
import numpy as np

import concourse.bacc as bacc
import concourse.mybir as mybir
import concourse.tile as tile
from concourse import bass, masks
from concourse.bass_utils import run_bass_kernel_spmd

B, NTOT, N, GS, C = 128, 129, 128, 16, 384
EPS = 0.05
RMS_EPS = 1e-6
NCORES = 8
SHB = B // NCORES            # batches per core (16)
PTS = SHB * N                # points per core (2048)
P = 128                      # partitions
TILES = PTS // P             # point-tiles per core (16); tile j == batch j
ROWS = B * N                 # gather table rows (16384)

F32 = mybir.dt.float32
BF16 = mybir.dt.bfloat16
I16 = mybir.dt.int16

NPE = 12                     # neighbor groups summed on the TensorEngine
NDVE = 1                     # neighbor products on DVE
NACT = 3                     # neighbor products on ACT
NNP = NDVE + NACT            # non-PE groups
assert NPE + NNP == GS

_CACHE = {}


def _build(knw_is_ones=True):
    # 64 KB/partition dynamic-DMA scratch -> 4096-descriptor SWDGE ring so
    # several 1024-descriptor gathers can be in flight.
    nc = bacc.Bacc(
        "TRN2", target_bir_lowering=False, debug=False,
        dynamic_dma_scratch_size=65536, num_swdge_queues=4,
    )

    xall = nc.dram_tensor("xall", [ROWS, C], BF16, kind="ExternalInput")
    xs = nc.dram_tensor("xs", [PTS, C], BF16, kind="ExternalInput")
    idxw = nc.dram_tensor("idxw", [P, PTS], I16, kind="ExternalInput")
    wnb_d = nc.dram_tensor("wnb", [P, TILES * GS], BF16, kind="ExternalInput")
    wnf_d = nc.dram_tensor("wnf", [P, TILES * GS], F32, kind="ExternalInput")
    rfx = nc.dram_tensor("rfx", [P, C], BF16, kind="ExternalInput")
    knw = nc.dram_tensor("knw", [P, C], BF16, kind="ExternalInput")
    # identity replicated NPE times along the free dim: contiguous in0 for
    # the dmat build (a doubly-broadcast tensor_tensor runs ~2x slower).
    identd = nc.dram_tensor("identd", [P, NPE * P], BF16, kind="ExternalInput")
    out = nc.dram_tensor("out", [PTS, C], BF16, kind="ExternalOutput")

    with tile.TileContext(nc) as tc:
        with (
            tc.tile_pool(name="consts", bufs=1) as cpool,
            tc.tile_pool(name="gbuf", bufs=5) as gpool,
            tc.tile_pool(name="work", bufs=4) as wpool,
            tc.tile_pool(name="small", bufs=6) as spool,
            tc.tile_pool(name="psum", bufs=8, space="PSUM") as ppool,
        ):
            # Spread input DMA issue across engine queues so they don't
            # serialize on the Sync sequencer, and order them so waiters
            # see their completion counts early: idx (gates the first
            # gather) alone on Sync; on Scalar the small consts come
            # before the big strided x load.
            # Warm the Q7 SWDGE gather ucode library (~7us load) with a
            # dummy 128-index gather of row 0 before anything depends on
            # it — the library reload is tied to the first DMAGatherAnt.
            dummy_idx = cpool.tile([P, 8], I16)
            nc.gpsimd.memset(dummy_idx[:], 0)
            dummy_out = cpool.tile([P, 1, C], BF16)
            nc.gpsimd.dma_gather(
                out_ap=dummy_out[:],
                in_ap=xall[:],
                idxs_ap=dummy_idx[:],
                num_idxs=P,
                num_idxs_reg=P,
                elem_size=C,
                queue_num=0,
            )

            idx_t = cpool.tile([P, PTS], I16)
            nc.sync.dma_start(idx_t[:], idxw[:])
            wnb_t = cpool.tile([P, TILES * GS], BF16)
            nc.scalar.dma_start(wnb_t[:], wnb_d[:])
            rfx_t = cpool.tile([P, C], BF16)
            nc.scalar.dma_start(rfx_t[:], rfx[:])
            identr = cpool.tile([P, NPE * P], BF16)
            nc.scalar.dma_start(identr[:], identd[:])
            wnf_t = cpool.tile([P, TILES * GS], F32)
            nc.scalar.dma_start(wnf_t[:], wnf_d[:])
            if not knw_is_ones:
                knw_t = cpool.tile([P, C], BF16)
                nc.scalar.dma_start(knw_t[:], knw[:])
            # all 16 x-tiles in one strided DMA: xt_all[p, j, :] = xs[j*128+p, :]
            # (last on this queue: its 2048-descriptor issue stalls later ones)
            xt_all = cpool.tile([P, TILES, C], BF16)
            nc.scalar.dma_start(
                xt_all[:], xs[:].rearrange("(j p) c -> p j c", p=P)
            )
            epsb = cpool.tile([P, 1], F32)
            nc.vector.memset(epsb[:], RMS_EPS)

            # fb = x + rfx for all 16 tiles in one shot (only needs xt_all)
            fb_all = cpool.tile([P, TILES, C], BF16)
            nc.vector.tensor_tensor(
                out=fb_all[:],
                in0=xt_all[:],
                in1=rfx_t[:].rearrange("p (x c) -> p x c", x=1).to_broadcast(
                    [P, TILES, C]
                ),
                op=mybir.AluOpType.add,
            )

            for j in range(TILES):
                rows = slice(j * P, (j + 1) * P)
                wcol = j * GS

                # gather all GS neighbors of this tile's 128 points:
                # nbr[p, g, :] = xall[idx[j, p, g], :] in bf16.
                # 2 x 1024-index gathers (>1024 faults the SWDGE ucode),
                # round-robined over the 4 SWDGE queues.
                nbr = gpool.tile([P, GS, C], BF16, tag="nbr")
                half = P * GS // 2
                for h in range(2):
                    nc.gpsimd.dma_gather(
                        out_ap=nbr[:, h * (GS // 2) : (h + 1) * (GS // 2), :],
                        in_ap=xall[:],
                        idxs_ap=idx_t[:, j * P + h * (half // 16) : j * P + (h + 1) * (half // 16)],
                        num_idxs=half,
                        num_idxs_reg=half,
                        elem_size=C,
                        queue_num=(2 * j + h) % 4,
                    )

                xt = xt_all[:, j, :]

                # dmat[p, g, q] = w[p, g] * (p == q): stationary diag
                # weights for the PE groups.
                dmat = wpool.tile([P, NPE, P], BF16, tag="dmat")
                nc.vector.tensor_tensor(
                    out=dmat[:],
                    in0=identr[:].rearrange("p (g c) -> p g c", g=NPE),
                    in1=wnb_t[:, wcol : wcol + NPE].to_broadcast([P, NPE, P]),
                    op=mybir.AluOpType.mult,
                )

                # non-PE neighbor products, all into one [P, NNP, C] tile:
                # DVE handles the first NDVE groups in one instruction,
                # ACT the remaining NACT as copy-with-scale.
                prod = wpool.tile([P, NNP, C], BF16, tag="prod")
                nc.vector.tensor_tensor(
                    out=prod[:, :NDVE, :],
                    in0=nbr[:, NPE : NPE + NDVE, :],
                    in1=wnb_t[:, wcol + NPE : wcol + NPE + NDVE].to_broadcast(
                        [P, NDVE, C]
                    ),
                    op=mybir.AluOpType.mult,
                )
                for m in range(NACT):
                    g = NPE + NDVE + m
                    nc.scalar.activation(
                        out=prod[:, NDVE + m, :], in_=nbr[:, g, :],
                        func=mybir.ActivationFunctionType.Copy,
                        scale=wnf_t[:, wcol + g : wcol + g + 1],
                    )
                # tree: 6 -> 3 -> 1, then subtract the center x (weights
                # sum to 1) so the partial re-enters PSUM with one matmul.
                h3 = NNP // 2
                nc.vector.tensor_tensor(
                    out=prod[:, 0:h3, :], in0=prod[:, 0:h3, :],
                    in1=prod[:, h3 : 2 * h3, :], op=mybir.AluOpType.add,
                )
                for m in range(1, h3):
                    nc.vector.tensor_tensor(
                        out=prod[:, 0, :], in0=prod[:, 0, :],
                        in1=prod[:, m, :], op=mybir.AluOpType.add,
                    )
                nc.vector.tensor_tensor(
                    out=prod[:, 0, :], in0=prod[:, 0, :], in1=xt,
                    op=mybir.AluOpType.subtract,
                )

                # PSUM: sf = sum_pe diag(w_g) @ nbr_g + I @ (sum_dve - x)
                acc_p = ppool.tile([P, C], F32, tag="acc")
                for g in range(NPE):
                    nc.tensor.matmul(
                        out=acc_p[:],
                        lhsT=dmat[:, g, :],
                        rhs=nbr[:, g, :],
                        start=(g == 0),
                        stop=False,
                    )
                nc.tensor.matmul(
                    out=acc_p[:], lhsT=identr[:, :P], rhs=prod[:, 0, :],
                    start=False, stop=True,
                )

                # rmsnorm on ACT straight out of PSUM:
                # rr = 1/sqrt(mean(sf^2) + eps); nt = sf * rr
                ssq = spool.tile([P, 1], F32, tag="ssq")
                sq = wpool.tile([P, C], BF16, tag="sq")
                nc.scalar.activation(
                    out=sq[:], in_=acc_p[:],
                    func=mybir.ActivationFunctionType.Square,
                    accum_out=ssq[:],
                )
                rms = spool.tile([P, 1], F32, tag="rms")
                nc.scalar.activation(
                    out=rms[:], in_=ssq[:],
                    func=mybir.ActivationFunctionType.Sqrt,
                    scale=1.0 / C, bias=epsb[:, :1],
                )
                rr = spool.tile([P, 1], F32, tag="rr")
                nc.vector.reciprocal(rr[:], rms[:])
                nt = wpool.tile([P, C], BF16, tag="nt")
                nc.scalar.activation(
                    out=nt[:], in_=acc_p[:],
                    func=mybir.ActivationFunctionType.Copy,
                    scale=rr[:, :1],
                )

                # out = nt*knw + fb; knw multiply skipped when all-1.
                if not knw_is_ones:
                    nc.vector.tensor_tensor(
                        out=nt[:], in0=nt[:], in1=knw_t[:], op=mybir.AluOpType.mult
                    )
                ob = wpool.tile([P, C], BF16, tag="ob")
                nc.vector.tensor_tensor(
                    out=ob[:], in0=fb_all[:, j, :], in1=nt[:],
                    op=mybir.AluOpType.add,
                )

                nc.sync.dma_start(out[rows, :], ob[:])

    nc.compile()
    return nc


def _get_nc(knw_is_ones=True):
    key = ("nc", knw_is_ones)
    if key not in _CACHE:
        _CACHE[key] = _build(knw_is_ones)
    return _CACHE[key]


def _wrap_idx(idx_core):
    """[PTS, GS] int -> [P, PTS] int16 wrapped layout for dma_gather.

    For tile j, half h (neighbors 8h..8h+7), gather-list position i
    (0..1023) lands in dst[i % 128, i // 128]; we want
    dst[p, g_h] = idx[j*128+p, 8h+g_h], so list[i] = blk[i % 128, 8h + i//128].
    The HW reads list[i] from idxs[i % 16, i // 16] over 16 partitions,
    and that [16, S] block must be replicated to all 128 partitions
    (each Q7 core reads its own copy).
    """
    out = np.zeros((P, PTS), np.int16)
    half = P * GS // 2                               # 1024
    S = half // 16                                   # 64
    for j in range(TILES):
        blk = idx_core[j * P : (j + 1) * P]          # [128, 16]
        for h in range(2):
            lst = blk[:, h * (GS // 2) : (h + 1) * (GS // 2)].T.reshape(-1)
            wrapped = lst.reshape(S, 16).T           # [16, 64]
            col = j * P + h * S
            out[:, col : col + S] = np.tile(wrapped, (P // 16, 1))
    return out


def _make_in_maps(inputs):
    f = np.asarray(inputs["f"], dtype=np.float32)
    distance = np.asarray(inputs["distance"], dtype=np.float32)
    rf = np.asarray(inputs["rf"], dtype=np.float32)
    knorm_w = np.asarray(inputs["knorm_w"], dtype=np.float32)
    idx_np = np.asarray(inputs["idx"]).astype(np.int64)

    x = np.ascontiguousarray(f[:, NTOT - N :, :].reshape(ROWS, C))
    x_bf = np.ascontiguousarray(x.astype(ml_dtypes.bfloat16))
    rfx_np = np.ascontiguousarray(rf[NTOT - N :][:P].astype(ml_dtypes.bfloat16))
    ident_np = np.ascontiguousarray(
        np.tile(np.eye(P, dtype=ml_dtypes.bfloat16), (1, NPE))
    )
    knw_np = np.ascontiguousarray(
        np.broadcast_to(knorm_w, (P, C)).astype(ml_dtypes.bfloat16)
    )

    # normalized idw weights, relaid out as [P, TILES*GS]:
    # wn[p, j*GS+g] = w[core batch j, point p, neighbor g]
    w = 1.0 / (distance + EPS)
    w = w / w.sum(axis=-1, keepdims=True)            # [B, N, GS] f32

    in_maps = []
    for c in range(NCORES):
        bs = slice(c * SHB, (c + 1) * SHB)
        idx_core = idx_np[bs].reshape(PTS, GS)
        wn_core = (
            w[bs].reshape(TILES, P, GS).transpose(1, 0, 2).reshape(P, TILES * GS)
        )
        in_maps.append(
            {
                "xall": x_bf,
                "xs": np.ascontiguousarray(x_bf[c * PTS : (c + 1) * PTS]),
                "idxw": _wrap_idx(idx_core),
                "wnb": np.ascontiguousarray(wn_core.astype(ml_dtypes.bfloat16)),
                "wnf": np.ascontiguousarray(wn_core),
                "rfx": rfx_np,
                "knw": knw_np,
                "identd": ident_np,
            }
        )
    return in_maps


def kernel(f, distance, rf, knorm_w, idx, **_unused):
    f = np.ascontiguousarray(np.asarray(f, dtype=np.float32))
    in_maps = _make_in_maps(
        {"f": f, "distance": distance, "rf": rf, "knorm_w": knorm_w, "idx": idx}
    )

    nc = _get_nc(bool(np.all(np.asarray(knorm_w) == 1.0)))
    res = run_bass_kernel_spmd(nc, in_maps, list(range(NCORES)))

    out = np.empty((B, NTOT, C), np.float32)
    out[:, : NTOT - N, :] = f[:, : NTOT - N, :]
    body = np.concatenate(
        [np.asarray(res.results[c]["out"]).astype(np.float32) for c in range(NCORES)],
        axis=0,
    )
    out[:, NTOT - N :, :] = body.reshape(B, N, C)
    return out
